# revision 34
# baseline (speedup 1.0000x reference)
"""Trainium2 Bass kernel for L0-regularized linear forward (hard-concrete gate).

Computes out[b,o] = sum_i x[b,i] * W[o,i] * z[b,o,i] + bias[o]
  where s = sigmoid((log(u) - log1p(-u) + log_alpha) / (2/3))
        z = clip(s * 1.2 - 0.1, 0, 1)

Shapes: x[32,2048] u[32,2048,2048] W[2048,2048] la[2048,2048] bias[2048]
Sharding: output-dim sharded, 2048/8 = 256 rows per core; no collectives.

Math used on device:
  y  = ln(u) - ln(1-u) + la;  yc = clamp(y, +-c), c = ln(11)/1.5
  z  = 1.2*sigmoid(1.5*yc) - 0.1              (exact: clip folds into clamp)
  sigmoid(1.5*y)-0.5 ~= p(y) = (c3*y^2 + c1)*y (deg-3, coefficients optimized
  for end-to-end variance against u~U(0,1); rel out err ~6e-3 vs 2e-2 gate)
  out = sum_i p(yc)*cb + 0.5*S0 + bias   with cb = 1.2*w*x, S0 = sum_i w*x

Engine schedule per (batch, half-of-256-rows) unit, all f16 after ACT:
  DMA : u half-tile (f32), broadcast row of x (f16)
  ACT : l2 = ln(1-u), l1 = ln(u)       (only the natural_log table -> 1 load)
  Pool: y1 = la - l2                   (scalar_tensor_tensor, eff 0.6)
  DVE : y = l1 + y1; custom op HC_GATE3_RED = clamp+poly3+mul-cb+accumulate
        in ONE instruction; cb = w12*xb (some offloaded to Pool)
  PE  : S0 row-sums of w*x (tiny f16 matmuls into PSUM)
"""

import sys
from contextlib import ExitStack

import numpy as np

if "/opt/trn_rl_repo" not in sys.path:
    sys.path.insert(0, "/opt/trn_rl_repo")

import concourse.bass as bass
import concourse.tile as tile
from concourse import bacc, mybir
from concourse.bass_utils import run_bass_kernel_spmd

F32 = mybir.dt.float32
F16 = mybir.dt.float16

B, OUT, IN = 32, 2048, 2048
N_CORES = 8
O_SH = OUT // N_CORES          # 256 output rows per core
H = O_SH // 128                # 2 partition-halves per core

C_CLAMP = float(np.log(11.0) / 1.5)
# variance-optimal deg-3: sigmoid(1.5 y) - 0.5 ~ (C3 y^2 + C1) y on [-c, c]
C1, C3 = 0.358500, -0.038292

N_CB_POOL = 3                  # of every 8 cb passes, this many go to Pool

_CACHE = {}


def _register_custom_op():
    """One fused DVE instruction:
        yc  = clamp(in0, -s0, s0)
        out = ((yc^2 * s1 + imm2) * yc) * in1
        accum_out = sum(out, free axis)
    8 ALU stages exactly (Zero-C0 is hoisted as stream-invariant)."""
    from concourse.dve_ops import (
        CUSTOM_DVE_SPECS,
        OPS,
        _CUSTOM_DVE_ROW_BASE,
        _SUB_OPCODE_FOR_NAME,
        DveOp,
    )
    from concourse.dve_spec import (
        AluOp, C0, Spec, Src0, Src1, Zero, lower, maxx, minn, sq,
    )
    from concourse.dve_spec import C1 as C1L, C2 as C2L
    from concourse.dve_table_gen import DveOpSpec

    name = "HC_GATE3_RED"
    if name in CUSTOM_DVE_SPECS:
        return next(o for o in OPS if o.name == name)

    yc = maxx(minn(Src0, C0), Zero - C0)
    body = ((sq(yc) * C1L + C2L) * yc) * Src1

    def _ref(in0, in1, s0, s1, imm2):
        x = np.clip(np.asarray(in0, np.float32), -s0, s0)
        w = np.asarray(in1, np.float32)
        out = ((x * x * s1 + imm2) * x) * w
        return out, out.sum(axis=-1, keepdims=True)

    spec = Spec(body=body, accum=AluOp.ADD, reference=_ref)
    row = _CUSTOM_DVE_ROW_BASE + len(OPS)
    _SUB_OPCODE_FOR_NAME[name] = row
    shas = {}
    for ver in ("v3", "v4"):
        dspec = DveOpSpec(name=name, opcode=row, uops=lower(spec, ver=ver),
                          rd1_en=True)
        shas[ver] = dspec.sha(ver)
    dve_op = DveOp(name, spec, subdim=False, uops_sha=shas)
    OPS.append(dve_op)
    CUSTOM_DVE_SPECS[name] = spec
    return dve_op


def _build_nc():
    if "nc" in _CACHE:
        return _CACHE["nc"]

    dve_op = _register_custom_op()

    nc = bacc.Bacc(
        "TRN2",
        target_bir_lowering=False,
        debug=False,
        num_devices=N_CORES,
    )
    u_d = nc.dram_tensor("u", [B, O_SH, IN], F32, kind="ExternalInput").ap()
    x16_d = nc.dram_tensor("x16", [B, IN], F16, kind="ExternalInput").ap()
    w12_d = nc.dram_tensor("w12", [O_SH, IN], F16, kind="ExternalInput").ap()
    la16_d = nc.dram_tensor("la16", [O_SH, IN], F16, kind="ExternalInput").ap()
    wt16_d = nc.dram_tensor("wt16", [IN, O_SH], F16, kind="ExternalInput").ap()
    xt16_d = nc.dram_tensor("xt16", [IN, B], F16, kind="ExternalInput").ap()
    bias_d = nc.dram_tensor("bias", [O_SH], F32, kind="ExternalInput").ap()
    out_d = nc.dram_tensor("out", [128, H, B], F32, kind="ExternalOutput").ap()

    with tile.TileContext(nc) as tc, ExitStack() as ctx:
        _kernel_body(ctx, tc, dve_op, u_d, x16_d, w12_d, la16_d, wt16_d,
                     xt16_d, bias_d, out_d)

    nc.compile()
    _CACHE["nc"] = nc
    return nc


def _bcast_row(ap_row):
    """[1, n] AP -> [128, n] AP with 0 partition stride."""
    return bass.AP(
        tensor=ap_row.tensor,
        offset=ap_row.offset,
        ap=[[0, 128], list(ap_row.ap[-1])],
    )


def _kernel_body(ctx, tc, dve_op, u_d, x16_d, w12_d, la16_d, wt16_d, xt16_d,
                 bias_d, out_d):
    nc = tc.nc
    Ln = mybir.ActivationFunctionType.Ln
    op = mybir.AluOpType

    singles = ctx.enter_context(tc.tile_pool(name="singles", bufs=1))

    # --- main loop pools (declared before constants so the first u/xb DMAs
    # are issued ahead of the setup DMAs on the in-order DMA queue) ---
    upool = ctx.enter_context(tc.tile_pool(name="u", bufs=3))
    l2pool = ctx.enter_context(tc.tile_pool(name="l2", bufs=2))
    ycpool = ctx.enter_context(tc.tile_pool(name="yc", bufs=2))
    xbpool = ctx.enter_context(tc.tile_pool(name="xb", bufs=2))
    cbpool = ctx.enter_context(tc.tile_pool(name="cb", bufs=4))
    junkpool = ctx.enter_context(tc.tile_pool(name="junk", bufs=1))

    # first working-set DMAs go out first; batch 0 is loaded per half so the
    # first ACT/DVE work starts after ~3us instead of ~6us of DMA
    ut0h = [upool.tile([128, IN], F32, name="uth") for h in range(H)]
    la16 = singles.tile([128, H, IN], F16)
    w12 = singles.tile([128, H, IN], F16)
    xb0 = xbpool.tile([128, IN], F16)
    la_v = la16_d.rearrange("(h p) i -> p h i", p=128)
    w12_v = w12_d.rearrange("(h p) i -> p h i", p=128)
    # dependency-ordered first transfers at quarter granularity: everything
    # the first (b=0, h=0, q=0) work needs, then successive quarters
    Q0 = IN // 2
    nc.sync.dma_start(out=ut0h[0][:, :Q0], in_=u_d[0, 0:128, :Q0])
    nc.sync.dma_start(out=la16[:, 0, :Q0], in_=la_v[:, 0, :Q0])
    nc.sync.dma_start(out=xb0[:, :Q0], in_=_bcast_row(x16_d[0:1, :Q0]))
    nc.sync.dma_start(out=w12[:, 0, :Q0], in_=w12_v[:, 0, :Q0])
    nc.sync.dma_start(out=ut0h[0][:, Q0:], in_=u_d[0, 0:128, Q0:])
    nc.sync.dma_start(out=la16[:, 0, Q0:], in_=la_v[:, 0, Q0:])
    nc.sync.dma_start(out=xb0[:, Q0:], in_=_bcast_row(x16_d[0:1, Q0:]))
    nc.sync.dma_start(out=w12[:, 0, Q0:], in_=w12_v[:, 0, Q0:])
    nc.sync.dma_start(out=ut0h[1], in_=u_d[0, 128:256, :])
    nc.sync.dma_start(out=la16[:, 1, :], in_=la_v[:, 1, :])
    nc.sync.dma_start(out=w12[:, 1, :], in_=w12_v[:, 1, :])
    bias_col = singles.tile([128, H], F32)
    nc.sync.dma_start(out=bias_col, in_=bias_d.rearrange("(h p) -> p h", p=128))

    # accumulator strip, column index = h*B + b
    accM = singles.tile([128, H * B], F32)
    accX = singles.tile([128, H], F32)      # b=0 second-column-half partials
    s0 = singles.tile([128, H * B], F32)

    def unit(b, uth, xb):
        # half-grain Ln for b==1 (its second u half is still in flight);
        # full-tile ops otherwise; y1/y-add always per-batch at 2x
        l2 = l2pool.tile([128, H, IN], F16)
        yc = ycpool.tile([128, H, IN], F16)
        if b == 1:
            for h in range(H):
                nc.scalar.activation(l2[:, h, :], uth[h], Ln,
                                     bias=1.0, scale=-1.0)
                nc.vector.tensor_sub(l2[:, h, :], la16[:, h, :], l2[:, h, :])
                nc.scalar.activation(yc[:, h, :], uth[h], Ln)
                nc.vector.tensor_add(yc[:, h, :], yc[:, h, :], l2[:, h, :])
        else:
            ut = uth[0]
            nc.scalar.activation(l2, ut, Ln, bias=1.0, scale=-1.0)  # ln(1-u)
            nc.vector.tensor_sub(l2, la16, l2)
            nc.scalar.activation(yc, ut, Ln)                        # ln(u)
            nc.vector.tensor_add(yc, yc, l2)
        for h in range(H):
            cb = cbpool.tile([128, IN], F16)
            nc.gpsimd.tensor_mul(cb, w12[:, h, :], xb)           # Pool
            junk = junkpool.tile([128, IN], F16)
            col = h * B + b
            nc.vector._custom_dve(
                dve_op,
                out=junk,
                in0=yc[:, h, :],
                in1=cb,
                s0=C_CLAMP, s1=C3, imm2=C1,
                accum_out=accM[:, col:col + 1],
            )

    def unit0(uth, xb):
        # batch 0 runs at quarter granularity so every engine starts ~5us
        # earlier during pipeline fill; the second column-half accumulates
        # into accX and is folded in at the combine
        Q = IN // 2
        l2 = l2pool.tile([128, H, IN], F16)
        yc = ycpool.tile([128, H, IN], F16)
        for h in range(H):
            cb = cbpool.tile([128, IN], F16)
            junk = junkpool.tile([128, IN], F16)
            for q in range(2):
                cs = slice(q * Q, (q + 1) * Q)
                nc.scalar.activation(l2[:, h, cs], uth[h][:, cs], Ln,
                                     bias=1.0, scale=-1.0)
                nc.vector.tensor_sub(l2[:, h, cs], la16[:, h, cs], l2[:, h, cs])
                nc.scalar.activation(yc[:, h, cs], uth[h][:, cs], Ln)
                nc.vector.tensor_add(yc[:, h, cs], yc[:, h, cs], l2[:, h, cs])
                nc.gpsimd.tensor_mul(cb[:, cs], w12[:, h, cs], xb[:, cs])
                acc = accM[:, h * B:h * B + 1] if q == 0 else accX[:, h:h + 1]
                nc.vector._custom_dve(
                    dve_op,
                    out=junk[:, cs],
                    in0=yc[:, h, cs],
                    in1=cb[:, cs],
                    s0=C_CLAMP, s1=C3, imm2=C1,
                    accum_out=acc,
                )

    for b in range(B):
        if b == 0:
            unit0(ut0h, xb0)
            continue
        xb = xbpool.tile([128, IN], F16)
        if b == 1:
            uth = [upool.tile([128, IN], F32, name="uth") for h in range(H)]
            nc.sync.dma_start(out=uth[0], in_=u_d[b, 0:128, :])
            nc.sync.dma_start(out=xb, in_=_bcast_row(x16_d[b:b + 1, :]))
            nc.sync.dma_start(out=uth[1], in_=u_d[b, 128:256, :])
        else:
            ut = upool.tile([128, H, IN], F32, name="utb", bufs=3)
            nc.sync.dma_start(out=ut, in_=u_d[b].rearrange("(h p) i -> p h i", p=128))
            nc.sync.dma_start(out=xb, in_=_bcast_row(x16_d[b:b + 1, :]))
            uth = [ut]
        unit(b, uth, xb)

    # --- S0 via PE: S0[o, b] = sum_i w[o,i] x[b,i] (feeds only the combine,
    # so it is emitted last and fills engine idle time near the tail) ---
    wt = singles.tile([128, IN // 128, O_SH], F16)
    nc.sync.dma_start(out=wt, in_=wt16_d.rearrange("(ki p) o -> p ki o", p=128))
    xt = singles.tile([128, IN // 128, B], F16)
    nc.sync.dma_start(out=xt, in_=xt16_d.rearrange("(ki p) b -> p ki b", p=128))
    with tc.psum_pool(name="ps", bufs=2) as psp:
        for h in range(H):
            osl = slice(h * 128, (h + 1) * 128)
            pm = psp.tile([128, B], F32)
            for ki in range(IN // 128):
                nc.tensor.matmul(pm, wt[:, ki, osl], xt[:, ki, :],
                                 start=(ki == 0), stop=(ki == IN // 128 - 1))
            nc.vector.tensor_copy(s0[:, h * B:(h + 1) * B], pm)

    # --- final combine: out = accM (+ accX for b=0) + 0.5*S0 + bias ---
    for h in range(H):
        nc.vector.tensor_add(accM[:, h * B:h * B + 1],
                             accM[:, h * B:h * B + 1], accX[:, h:h + 1])
    comb = singles.tile([128, H * B], F32)
    nc.vector.scalar_tensor_tensor(out=comb, in0=s0, scalar=0.5,
                                   in1=accM, op0=op.mult, op1=op.add)
    for h in range(H):
        nc.vector.tensor_scalar(
            comb[:, h * B:(h + 1) * B], comb[:, h * B:(h + 1) * B],
            bias_col[:, h:h + 1], None, op.add,
        )
    out_v = out_d.rearrange("p h b -> p (h b)")
    nc.sync.dma_start(out=out_v, in_=comb)


def kernel(x, u, weight, log_alpha, bias):
    x = np.ascontiguousarray(x, dtype=np.float32)
    u = np.ascontiguousarray(u, dtype=np.float32)
    weight = np.ascontiguousarray(weight, dtype=np.float32)
    log_alpha = np.ascontiguousarray(log_alpha, dtype=np.float32)
    bias = np.ascontiguousarray(bias, dtype=np.float32)

    nc = _build_nc()

    x16 = x.astype(np.float16)
    in_maps = []
    for c in range(N_CORES):
        sl = slice(c * O_SH, (c + 1) * O_SH)
        wsl = weight[sl]
        in_maps.append(
            {
                "u": np.ascontiguousarray(u[:, sl, :]),
                "x16": x16,
                "w12": np.ascontiguousarray((1.2 * wsl).astype(np.float16)),
                "la16": np.ascontiguousarray(log_alpha[sl].astype(np.float16)),
                "wt16": np.ascontiguousarray(wsl.T.astype(np.float16)),
                "xt16": np.ascontiguousarray(x.T.astype(np.float16)),
                "bias": np.ascontiguousarray(bias[sl]),
            }
        )

    import os

    trace = bool(int(os.environ.get("KERNEL_TRACE", "0")))
    res = run_bass_kernel_spmd(
        nc, in_maps, core_ids=list(range(N_CORES)), trace=trace
    )
    kernel._last = res

    out = np.empty((B, OUT), dtype=np.float32)
    for c in range(N_CORES):
        oc = res.results[c]["out"]          # [128, H, B]
        out[:, c * O_SH:(c + 1) * O_SH] = oc.transpose(2, 1, 0).reshape(B, O_SH)
    return out


# revision 41
# speedup vs baseline: 1.0057x; 1.0057x over previous
"""Trainium2 Bass kernel for L0-regularized linear forward (hard-concrete gate).

Computes out[b,o] = sum_i x[b,i] * W[o,i] * z[b,o,i] + bias[o]
  where s = sigmoid((log(u) - log1p(-u) + log_alpha) / (2/3))
        z = clip(s * 1.2 - 0.1, 0, 1)

Shapes: x[32,2048] u[32,2048,2048] W[2048,2048] la[2048,2048] bias[2048]
Sharding: output-dim sharded, 2048/8 = 256 rows per core; no collectives.

Math used on device:
  y  = ln(u) - ln(1-u) + la;  yc = clamp(y, +-c), c = ln(11)/1.5
  z  = 1.2*sigmoid(1.5*yc) - 0.1              (exact: clip folds into clamp)
  sigmoid(1.5*y)-0.5 ~= p(y) = (c3*y^2 + c1)*y (deg-3, coefficients optimized
  for end-to-end variance against u~U(0,1); rel out err ~6e-3 vs 2e-2 gate)
  out = sum_i p(yc)*cb + 0.5*S0 + bias   with cb = 1.2*w*x, S0 = sum_i w*x

Engine schedule per (batch, half-of-256-rows) unit, all f16 after ACT:
  DMA : u half-tile (f32), broadcast row of x (f16)
  ACT : l2 = ln(1-u), l1 = ln(u)       (only the natural_log table -> 1 load)
  Pool: y1 = la - l2                   (scalar_tensor_tensor, eff 0.6)
  DVE : y = l1 + y1; custom op HC_GATE3_RED = clamp+poly3+mul-cb+accumulate
        in ONE instruction; cb = w12*xb (some offloaded to Pool)
  PE  : S0 row-sums of w*x (tiny f16 matmuls into PSUM)
"""

import sys
from contextlib import ExitStack

import numpy as np

if "/opt/trn_rl_repo" not in sys.path:
    sys.path.insert(0, "/opt/trn_rl_repo")

import concourse.bass as bass
import concourse.tile as tile
from concourse import bacc, mybir
from concourse.bass_utils import run_bass_kernel_spmd

F32 = mybir.dt.float32
F16 = mybir.dt.float16

B, OUT, IN = 32, 2048, 2048
N_CORES = 8
O_SH = OUT // N_CORES          # 256 output rows per core
H = O_SH // 128                # 2 partition-halves per core

C_CLAMP = float(np.log(11.0) / 1.5)
# variance-optimal deg-3: sigmoid(1.5 y) - 0.5 ~ (C3 y^2 + C1) y on [-c, c]
C1, C3 = 0.358500, -0.038292

N_CB_POOL = 3                  # of every 8 cb passes, this many go to Pool

_CACHE = {}


def _register_custom_op():
    """One fused DVE instruction:
        yc  = clamp(in0, -s0, s0)
        out = ((yc^2 * s1 + imm2) * yc) * in1
        accum_out = sum(out, free axis)
    8 ALU stages exactly (Zero-C0 is hoisted as stream-invariant)."""
    from concourse.dve_ops import (
        CUSTOM_DVE_SPECS,
        OPS,
        _CUSTOM_DVE_ROW_BASE,
        _SUB_OPCODE_FOR_NAME,
        DveOp,
    )
    from concourse.dve_spec import (
        AluOp, C0, Spec, Src0, Src1, Zero, lower, maxx, minn, sq,
    )
    from concourse.dve_spec import C1 as C1L, C2 as C2L
    from concourse.dve_table_gen import DveOpSpec

    name = "HC_GATE3_RED"
    if name in CUSTOM_DVE_SPECS:
        return next(o for o in OPS if o.name == name)

    yc = maxx(minn(Src0, C0), Zero - C0)
    body = ((sq(yc) * C1L + C2L) * yc) * Src1

    def _ref(in0, in1, s0, s1, imm2):
        x = np.clip(np.asarray(in0, np.float32), -s0, s0)
        w = np.asarray(in1, np.float32)
        out = ((x * x * s1 + imm2) * x) * w
        return out, out.sum(axis=-1, keepdims=True)

    spec = Spec(body=body, accum=AluOp.ADD, reference=_ref)
    row = _CUSTOM_DVE_ROW_BASE + len(OPS)
    _SUB_OPCODE_FOR_NAME[name] = row
    shas = {}
    for ver in ("v3", "v4"):
        dspec = DveOpSpec(name=name, opcode=row, uops=lower(spec, ver=ver),
                          rd1_en=True)
        shas[ver] = dspec.sha(ver)
    dve_op = DveOp(name, spec, subdim=False, uops_sha=shas)
    OPS.append(dve_op)
    CUSTOM_DVE_SPECS[name] = spec
    return dve_op


def _build_nc():
    if "nc" in _CACHE:
        return _CACHE["nc"]

    dve_op = _register_custom_op()

    nc = bacc.Bacc(
        "TRN2",
        target_bir_lowering=False,
        debug=False,
        num_devices=N_CORES,
    )
    u_d = nc.dram_tensor("u", [B, O_SH, IN], F32, kind="ExternalInput").ap()
    x16_d = nc.dram_tensor("x16", [B, IN], F16, kind="ExternalInput").ap()
    w12_d = nc.dram_tensor("w12", [O_SH, IN], F16, kind="ExternalInput").ap()
    la16_d = nc.dram_tensor("la16", [O_SH, IN], F16, kind="ExternalInput").ap()
    wt16_d = nc.dram_tensor("wt16", [IN, O_SH], F16, kind="ExternalInput").ap()
    xt16_d = nc.dram_tensor("xt16", [IN, B], F16, kind="ExternalInput").ap()
    bias_d = nc.dram_tensor("bias", [O_SH], F32, kind="ExternalInput").ap()
    out_d = nc.dram_tensor("out", [128, H, B], F32, kind="ExternalOutput").ap()

    with tile.TileContext(nc) as tc, ExitStack() as ctx:
        _kernel_body(ctx, tc, dve_op, u_d, x16_d, w12_d, la16_d, wt16_d,
                     xt16_d, bias_d, out_d)

    nc.compile()
    _CACHE["nc"] = nc
    return nc


def _bcast_row(ap_row):
    """[1, n] AP -> [128, n] AP with 0 partition stride."""
    return bass.AP(
        tensor=ap_row.tensor,
        offset=ap_row.offset,
        ap=[[0, 128], list(ap_row.ap[-1])],
    )


def _kernel_body(ctx, tc, dve_op, u_d, x16_d, w12_d, la16_d, wt16_d, xt16_d,
                 bias_d, out_d):
    nc = tc.nc
    Ln = mybir.ActivationFunctionType.Ln
    op = mybir.AluOpType

    singles = ctx.enter_context(tc.tile_pool(name="singles", bufs=1))

    # --- main loop pools (declared before constants so the first u/xb DMAs
    # are issued ahead of the setup DMAs on the in-order DMA queue) ---
    upool = ctx.enter_context(tc.tile_pool(name="u", bufs=3))
    l2pool = ctx.enter_context(tc.tile_pool(name="l2", bufs=2))
    ycpool = ctx.enter_context(tc.tile_pool(name="yc", bufs=2))
    xbpool = ctx.enter_context(tc.tile_pool(name="xb", bufs=2))
    cbpool = ctx.enter_context(tc.tile_pool(name="cb", bufs=4))
    junkpool = ctx.enter_context(tc.tile_pool(name="junk", bufs=2))

    # first working-set DMAs go out first; batch 0 is loaded per half so the
    # first ACT/DVE work starts after ~3us instead of ~6us of DMA
    ut0h = [upool.tile([128, IN], F32, name="uth") for h in range(H)]
    la16 = singles.tile([128, H, IN], F16)
    w12 = singles.tile([128, H, IN], F16)
    xb0 = xbpool.tile([128, IN], F16)
    la_v = la16_d.rearrange("(h p) i -> p h i", p=128)
    w12_v = w12_d.rearrange("(h p) i -> p h i", p=128)
    # dependency-ordered first transfers at quarter granularity: everything
    # the first (b=0, h=0, q=0) work needs, then successive quarters
    Q0 = IN // 2
    nc.sync.dma_start(out=ut0h[0][:, :Q0], in_=u_d[0, 0:128, :Q0])
    nc.sync.dma_start(out=la16[:, 0, :Q0], in_=la_v[:, 0, :Q0])
    nc.sync.dma_start(out=xb0[:, :Q0], in_=_bcast_row(x16_d[0:1, :Q0]))
    nc.sync.dma_start(out=w12[:, 0, :Q0], in_=w12_v[:, 0, :Q0])
    nc.sync.dma_start(out=ut0h[0][:, Q0:], in_=u_d[0, 0:128, Q0:])
    nc.sync.dma_start(out=la16[:, 0, Q0:], in_=la_v[:, 0, Q0:])
    nc.sync.dma_start(out=xb0[:, Q0:], in_=_bcast_row(x16_d[0:1, Q0:]))
    nc.sync.dma_start(out=w12[:, 0, Q0:], in_=w12_v[:, 0, Q0:])
    nc.sync.dma_start(out=ut0h[1], in_=u_d[0, 128:256, :])
    nc.sync.dma_start(out=la16[:, 1, :], in_=la_v[:, 1, :])
    nc.sync.dma_start(out=w12[:, 1, :], in_=w12_v[:, 1, :])
    bias_col = singles.tile([128, H], F32)
    nc.sync.dma_start(out=bias_col, in_=bias_d.rearrange("(h p) -> p h", p=128))

    # accumulator strip, column index = h*B + b
    accM = singles.tile([128, H * B], F32)
    accX = singles.tile([128, H], F32)      # b=0 second-column-half partials
    s0 = singles.tile([128, H * B], F32)

    def unit(b, uth, xb):
        # half-grain Ln for b==1 (its second u half is still in flight);
        # full-tile ops otherwise; y1/y-add always per-batch at 2x
        l2 = l2pool.tile([128, H, IN], F16)
        yc = ycpool.tile([128, H, IN], F16)
        if b == 1:
            for h in range(H):
                nc.scalar.activation(l2[:, h, :], uth[h], Ln,
                                     bias=1.0, scale=-1.0)
                nc.vector.tensor_sub(l2[:, h, :], la16[:, h, :], l2[:, h, :])
                nc.scalar.activation(yc[:, h, :], uth[h], Ln)
                nc.vector.tensor_add(yc[:, h, :], yc[:, h, :], l2[:, h, :])
        else:
            ut = uth[0]
            nc.scalar.activation(l2, ut, Ln, bias=1.0, scale=-1.0)  # ln(1-u)
            nc.vector.tensor_sub(l2, la16, l2)
            nc.scalar.activation(yc, ut, Ln)                        # ln(u)
            nc.vector.tensor_add(yc, yc, l2)
        for h in range(H):
            cb = cbpool.tile([128, IN], F16)
            nc.gpsimd.tensor_mul(cb, w12[:, h, :], xb)           # Pool
            junk = junkpool.tile([128, IN], F16)
            col = h * B + b
            nc.vector._custom_dve(
                dve_op,
                out=junk,
                in0=yc[:, h, :],
                in1=cb,
                s0=C_CLAMP, s1=C3, imm2=C1,
                accum_out=accM[:, col:col + 1],
            )

    def unit0(uth, xb):
        # batch 0 runs at quarter granularity so every engine starts ~5us
        # earlier during pipeline fill; the second column-half accumulates
        # into accX and is folded in at the combine
        Q = IN // 2
        l2 = l2pool.tile([128, H, IN], F16)
        yc = ycpool.tile([128, H, IN], F16)
        for h in range(H):
            cb = cbpool.tile([128, IN], F16)
            junk = junkpool.tile([128, IN], F16)
            for q in range(2):
                cs = slice(q * Q, (q + 1) * Q)
                nc.scalar.activation(l2[:, h, cs], uth[h][:, cs], Ln,
                                     bias=1.0, scale=-1.0)
                nc.vector.tensor_sub(l2[:, h, cs], la16[:, h, cs], l2[:, h, cs])
                nc.scalar.activation(yc[:, h, cs], uth[h][:, cs], Ln)
                nc.vector.tensor_add(yc[:, h, cs], yc[:, h, cs], l2[:, h, cs])
                nc.gpsimd.tensor_mul(cb[:, cs], w12[:, h, cs], xb[:, cs])
                acc = accM[:, h * B:h * B + 1] if q == 0 else accX[:, h:h + 1]
                nc.vector._custom_dve(
                    dve_op,
                    out=junk[:, cs],
                    in0=yc[:, h, cs],
                    in1=cb[:, cs],
                    s0=C_CLAMP, s1=C3, imm2=C1,
                    accum_out=acc,
                )

    for b in range(B):
        if b == 0:
            unit0(ut0h, xb0)
            continue
        xb = xbpool.tile([128, IN], F16)
        if b == 1:
            uth = [upool.tile([128, IN], F32, name="uth") for h in range(H)]
            nc.sync.dma_start(out=uth[0], in_=u_d[b, 0:128, :])
            nc.sync.dma_start(out=xb, in_=_bcast_row(x16_d[b:b + 1, :]))
            nc.sync.dma_start(out=uth[1], in_=u_d[b, 128:256, :])
        else:
            ut = upool.tile([128, H, IN], F32, name="utb", bufs=3)
            nc.sync.dma_start(out=ut, in_=u_d[b].rearrange("(h p) i -> p h i", p=128))
            nc.sync.dma_start(out=xb, in_=_bcast_row(x16_d[b:b + 1, :]))
            uth = [ut]
        unit(b, uth, xb)

    # --- S0 via PE: S0[o, b] = sum_i w[o,i] x[b,i] (feeds only the combine,
    # so it is emitted last and fills engine idle time near the tail) ---
    wt = singles.tile([128, IN // 128, O_SH], F16)
    nc.sync.dma_start(out=wt, in_=wt16_d.rearrange("(ki p) o -> p ki o", p=128))
    xt = singles.tile([128, IN // 128, B], F16)
    nc.sync.dma_start(out=xt, in_=xt16_d.rearrange("(ki p) b -> p ki b", p=128))
    with tc.psum_pool(name="ps", bufs=2) as psp:
        for h in range(H):
            osl = slice(h * 128, (h + 1) * 128)
            pm = psp.tile([128, B], F32)
            for ki in range(IN // 128):
                nc.tensor.matmul(pm, wt[:, ki, osl], xt[:, ki, :],
                                 start=(ki == 0), stop=(ki == IN // 128 - 1))
            nc.vector.tensor_copy(s0[:, h * B:(h + 1) * B], pm)

    # --- final combine: out = accM (+ accX for b=0) + 0.5*S0 + bias ---
    for h in range(H):
        nc.vector.tensor_add(accM[:, h * B:h * B + 1],
                             accM[:, h * B:h * B + 1], accX[:, h:h + 1])
    comb = singles.tile([128, H * B], F32)
    nc.vector.scalar_tensor_tensor(out=comb, in0=s0, scalar=0.5,
                                   in1=accM, op0=op.mult, op1=op.add)
    for h in range(H):
        nc.vector.tensor_scalar(
            comb[:, h * B:(h + 1) * B], comb[:, h * B:(h + 1) * B],
            bias_col[:, h:h + 1], None, op.add,
        )
    out_v = out_d.rearrange("p h b -> p (h b)")
    nc.sync.dma_start(out=out_v, in_=comb)


def kernel(x, u, weight, log_alpha, bias):
    x = np.ascontiguousarray(x, dtype=np.float32)
    u = np.ascontiguousarray(u, dtype=np.float32)
    weight = np.ascontiguousarray(weight, dtype=np.float32)
    log_alpha = np.ascontiguousarray(log_alpha, dtype=np.float32)
    bias = np.ascontiguousarray(bias, dtype=np.float32)

    nc = _build_nc()

    x16 = x.astype(np.float16)
    in_maps = []
    for c in range(N_CORES):
        sl = slice(c * O_SH, (c + 1) * O_SH)
        wsl = weight[sl]
        in_maps.append(
            {
                "u": np.ascontiguousarray(u[:, sl, :]),
                "x16": x16,
                "w12": np.ascontiguousarray((1.2 * wsl).astype(np.float16)),
                "la16": np.ascontiguousarray(log_alpha[sl].astype(np.float16)),
                "wt16": np.ascontiguousarray(wsl.T.astype(np.float16)),
                "xt16": np.ascontiguousarray(x.T.astype(np.float16)),
                "bias": np.ascontiguousarray(bias[sl]),
            }
        )

    import os

    trace = bool(int(os.environ.get("KERNEL_TRACE", "0")))
    res = run_bass_kernel_spmd(
        nc, in_maps, core_ids=list(range(N_CORES)), trace=trace
    )
    kernel._last = res

    out = np.empty((B, OUT), dtype=np.float32)
    for c in range(N_CORES):
        oc = res.results[c]["out"]          # [128, H, B]
        out[:, c * O_SH:(c + 1) * O_SH] = oc.transpose(2, 1, 0).reshape(B, O_SH)
    return out


# revision 42
# speedup vs baseline: 1.0060x; 1.0003x over previous
"""Trainium2 Bass kernel for L0-regularized linear forward (hard-concrete gate).

Computes out[b,o] = sum_i x[b,i] * W[o,i] * z[b,o,i] + bias[o]
  where s = sigmoid((log(u) - log1p(-u) + log_alpha) / (2/3))
        z = clip(s * 1.2 - 0.1, 0, 1)

Shapes: x[32,2048] u[32,2048,2048] W[2048,2048] la[2048,2048] bias[2048]
Sharding: output-dim sharded, 2048/8 = 256 rows per core; no collectives.

Math used on device:
  y  = ln(u) - ln(1-u) + la;  yc = clamp(y, +-c), c = ln(11)/1.5
  z  = 1.2*sigmoid(1.5*yc) - 0.1              (exact: clip folds into clamp)
  sigmoid(1.5*y)-0.5 ~= p(y) = (c3*y^2 + c1)*y (deg-3, coefficients optimized
  for end-to-end variance against u~U(0,1); rel out err ~6e-3 vs 2e-2 gate)
  out = sum_i p(yc)*cb + 0.5*S0 + bias   with cb = 1.2*w*x, S0 = sum_i w*x

Engine schedule per (batch, half-of-256-rows) unit, all f16 after ACT:
  DMA : u half-tile (f32), broadcast row of x (f16)
  ACT : l2 = ln(1-u), l1 = ln(u)       (only the natural_log table -> 1 load)
  Pool: y1 = la - l2                   (scalar_tensor_tensor, eff 0.6)
  DVE : y = l1 + y1; custom op HC_GATE3_RED = clamp+poly3+mul-cb+accumulate
        in ONE instruction; cb = w12*xb (some offloaded to Pool)
  PE  : S0 row-sums of w*x (tiny f16 matmuls into PSUM)
"""

import sys
from contextlib import ExitStack

import numpy as np

if "/opt/trn_rl_repo" not in sys.path:
    sys.path.insert(0, "/opt/trn_rl_repo")

import concourse.bass as bass
import concourse.tile as tile
from concourse import bacc, mybir
from concourse.bass_utils import run_bass_kernel_spmd

F32 = mybir.dt.float32
F16 = mybir.dt.float16

B, OUT, IN = 32, 2048, 2048
N_CORES = 8
O_SH = OUT // N_CORES          # 256 output rows per core
H = O_SH // 128                # 2 partition-halves per core

C_CLAMP = float(np.log(11.0) / 1.5)
# variance-optimal deg-3: sigmoid(1.5 y) - 0.5 ~ (C3 y^2 + C1) y on [-c, c]
C1, C3 = 0.358500, -0.038292

N_CB_POOL = 3                  # of every 8 cb passes, this many go to Pool

_CACHE = {}


def _register_custom_op():
    """One fused DVE instruction:
        yc  = clamp(in0, -s0, s0)
        out = ((yc^2 * s1 + imm2) * yc) * in1
        accum_out = sum(out, free axis)
    8 ALU stages exactly (Zero-C0 is hoisted as stream-invariant)."""
    from concourse.dve_ops import (
        CUSTOM_DVE_SPECS,
        OPS,
        _CUSTOM_DVE_ROW_BASE,
        _SUB_OPCODE_FOR_NAME,
        DveOp,
    )
    from concourse.dve_spec import (
        AluOp, C0, Spec, Src0, Src1, Zero, lower, maxx, minn, sq,
    )
    from concourse.dve_spec import C1 as C1L, C2 as C2L
    from concourse.dve_table_gen import DveOpSpec

    name = "HC_GATE3_RED"
    if name in CUSTOM_DVE_SPECS:
        return next(o for o in OPS if o.name == name)

    yc = maxx(minn(Src0, C0), Zero - C0)
    body = ((sq(yc) * C1L + C2L) * yc) * Src1

    def _ref(in0, in1, s0, s1, imm2):
        x = np.clip(np.asarray(in0, np.float32), -s0, s0)
        w = np.asarray(in1, np.float32)
        out = ((x * x * s1 + imm2) * x) * w
        return out, out.sum(axis=-1, keepdims=True)

    spec = Spec(body=body, accum=AluOp.ADD, reference=_ref)
    row = _CUSTOM_DVE_ROW_BASE + len(OPS)
    _SUB_OPCODE_FOR_NAME[name] = row
    shas = {}
    for ver in ("v3", "v4"):
        dspec = DveOpSpec(name=name, opcode=row, uops=lower(spec, ver=ver),
                          rd1_en=True)
        shas[ver] = dspec.sha(ver)
    dve_op = DveOp(name, spec, subdim=False, uops_sha=shas)
    OPS.append(dve_op)
    CUSTOM_DVE_SPECS[name] = spec
    return dve_op


def _build_nc():
    if "nc" in _CACHE:
        return _CACHE["nc"]

    dve_op = _register_custom_op()

    nc = bacc.Bacc(
        "TRN2",
        target_bir_lowering=False,
        debug=False,
        num_devices=N_CORES,
    )
    u_d = nc.dram_tensor("u", [B, O_SH, IN], F32, kind="ExternalInput").ap()
    x16_d = nc.dram_tensor("x16", [B, IN], F16, kind="ExternalInput").ap()
    w12_d = nc.dram_tensor("w12", [O_SH, IN], F16, kind="ExternalInput").ap()
    la16_d = nc.dram_tensor("la16", [O_SH, IN], F16, kind="ExternalInput").ap()
    wt16_d = nc.dram_tensor("wt16", [IN, O_SH], F16, kind="ExternalInput").ap()
    xt16_d = nc.dram_tensor("xt16", [IN, B], F16, kind="ExternalInput").ap()
    bias_d = nc.dram_tensor("bias", [O_SH], F32, kind="ExternalInput").ap()
    out_d = nc.dram_tensor("out", [128, H, B], F32, kind="ExternalOutput").ap()

    with tile.TileContext(nc) as tc, ExitStack() as ctx:
        _kernel_body(ctx, tc, dve_op, u_d, x16_d, w12_d, la16_d, wt16_d,
                     xt16_d, bias_d, out_d)

    nc.compile()
    _CACHE["nc"] = nc
    return nc


def _bcast_row(ap_row):
    """[1, n] AP -> [128, n] AP with 0 partition stride."""
    return bass.AP(
        tensor=ap_row.tensor,
        offset=ap_row.offset,
        ap=[[0, 128], list(ap_row.ap[-1])],
    )


def _kernel_body(ctx, tc, dve_op, u_d, x16_d, w12_d, la16_d, wt16_d, xt16_d,
                 bias_d, out_d):
    nc = tc.nc
    Ln = mybir.ActivationFunctionType.Ln
    op = mybir.AluOpType

    singles = ctx.enter_context(tc.tile_pool(name="singles", bufs=1))

    # --- main loop pools (declared before constants so the first u/xb DMAs
    # are issued ahead of the setup DMAs on the in-order DMA queue) ---
    upool = ctx.enter_context(tc.tile_pool(name="u", bufs=3))
    l2pool = ctx.enter_context(tc.tile_pool(name="l2", bufs=2))
    ycpool = ctx.enter_context(tc.tile_pool(name="yc", bufs=2))
    xbpool = ctx.enter_context(tc.tile_pool(name="xb", bufs=2))
    cbpool = ctx.enter_context(tc.tile_pool(name="cb", bufs=4))
    junkpool = ctx.enter_context(tc.tile_pool(name="junk", bufs=2))

    # first working-set DMAs go out first; batch 0 is loaded per half so the
    # first ACT/DVE work starts after ~3us instead of ~6us of DMA
    ut0h = [upool.tile([128, IN], F32, name="uth") for h in range(H)]
    la16 = singles.tile([128, H, IN], F16)
    w12 = singles.tile([128, H, IN], F16)
    xb0 = xbpool.tile([128, IN], F16)
    la_v = la16_d.rearrange("(h p) i -> p h i", p=128)
    w12_v = w12_d.rearrange("(h p) i -> p h i", p=128)
    # dependency-ordered first transfers at quarter granularity: everything
    # the first (b=0, h=0, q=0) work needs, then successive quarters
    Q0 = IN // 2
    nc.sync.dma_start(out=ut0h[0][:, :Q0], in_=u_d[0, 0:128, :Q0])
    nc.sync.dma_start(out=la16[:, 0, :Q0], in_=la_v[:, 0, :Q0])
    nc.sync.dma_start(out=xb0[:, :Q0], in_=_bcast_row(x16_d[0:1, :Q0]))
    nc.sync.dma_start(out=w12[:, 0, :Q0], in_=w12_v[:, 0, :Q0])
    nc.sync.dma_start(out=ut0h[0][:, Q0:], in_=u_d[0, 0:128, Q0:])
    nc.sync.dma_start(out=la16[:, 0, Q0:], in_=la_v[:, 0, Q0:])
    nc.sync.dma_start(out=xb0[:, Q0:], in_=_bcast_row(x16_d[0:1, Q0:]))
    nc.sync.dma_start(out=w12[:, 0, Q0:], in_=w12_v[:, 0, Q0:])
    nc.sync.dma_start(out=ut0h[1], in_=u_d[0, 128:256, :])
    nc.sync.dma_start(out=la16[:, 1, :], in_=la_v[:, 1, :])
    nc.sync.dma_start(out=w12[:, 1, :], in_=w12_v[:, 1, :])
    bias_col = singles.tile([128, H], F32)
    nc.sync.dma_start(out=bias_col, in_=bias_d.rearrange("(h p) -> p h", p=128))

    # accumulator strip, column index = h*B + b
    accM = singles.tile([128, H * B], F32)
    accX = singles.tile([128, H], F32)      # b=0 second-column-half partials
    s0 = singles.tile([128, H * B], F32)

    def unit(b, uth, xb):
        # half-grain Ln for b==1 (its second u half is still in flight);
        # full-tile ops otherwise; y1/y-add always per-batch at 2x
        l2 = l2pool.tile([128, H, IN], F16)
        yc = ycpool.tile([128, H, IN], F16)
        if b == 1:
            for h in range(H):
                nc.scalar.activation(l2[:, h, :], uth[h], Ln,
                                     bias=1.0, scale=-1.0)
                nc.vector.tensor_sub(l2[:, h, :], la16[:, h, :], l2[:, h, :])
                nc.scalar.activation(yc[:, h, :], uth[h], Ln)
                nc.vector.tensor_add(yc[:, h, :], yc[:, h, :], l2[:, h, :])
        else:
            ut = uth[0]
            nc.scalar.activation(l2, ut, Ln, bias=1.0, scale=-1.0)  # ln(1-u)
            nc.vector.tensor_sub(l2, la16, l2)
            nc.scalar.activation(yc, ut, Ln)                        # ln(u)
            nc.vector.tensor_add(yc, yc, l2)
        for h in range(H):
            cb = cbpool.tile([128, IN], F16)
            nc.gpsimd.tensor_mul(cb, w12[:, h, :], xb)           # Pool
            junk = junkpool.tile([128, IN], F16)
            col = h * B + b
            nc.vector._custom_dve(
                dve_op,
                out=junk,
                in0=yc[:, h, :],
                in1=cb,
                s0=C_CLAMP, s1=C3, imm2=C1,
                accum_out=accM[:, col:col + 1],
            )

    def unit0(uth, xb):
        # batch 0 runs at quarter granularity so every engine starts ~5us
        # earlier during pipeline fill; the second column-half accumulates
        # into accX and is folded in at the combine
        Q = IN // 2
        l2 = l2pool.tile([128, H, IN], F16)
        yc = ycpool.tile([128, H, IN], F16)
        for h in range(H):
            cb = cbpool.tile([128, IN], F16)
            junk = junkpool.tile([128, IN], F16)
            for q in range(2):
                cs = slice(q * Q, (q + 1) * Q)
                nc.scalar.activation(l2[:, h, cs], uth[h][:, cs], Ln,
                                     bias=1.0, scale=-1.0)
                nc.vector.tensor_sub(l2[:, h, cs], la16[:, h, cs], l2[:, h, cs])
                nc.scalar.activation(yc[:, h, cs], uth[h][:, cs], Ln)
                nc.vector.tensor_add(yc[:, h, cs], yc[:, h, cs], l2[:, h, cs])
                nc.gpsimd.tensor_mul(cb[:, cs], w12[:, h, cs], xb[:, cs])
                acc = accM[:, h * B:h * B + 1] if q == 0 else accX[:, h:h + 1]
                nc.vector._custom_dve(
                    dve_op,
                    out=junk[:, cs],
                    in0=yc[:, h, cs],
                    in1=cb[:, cs],
                    s0=C_CLAMP, s1=C3, imm2=C1,
                    accum_out=acc,
                )

    for b in range(B):
        if b == 0:
            unit0(ut0h, xb0)
            continue
        xb = xbpool.tile([128, IN], F16)
        if b == 1:
            uth = [upool.tile([128, IN], F32, name="uth") for h in range(H)]
            nc.sync.dma_start(out=uth[0], in_=u_d[b, 0:128, :])
            nc.sync.dma_start(out=xb, in_=_bcast_row(x16_d[b:b + 1, :]))
            nc.sync.dma_start(out=uth[1], in_=u_d[b, 128:256, :])
        else:
            ut = upool.tile([128, H, IN], F32, name="utb", bufs=3)
            nc.sync.dma_start(out=ut, in_=u_d[b].rearrange("(h p) i -> p h i", p=128))
            nc.sync.dma_start(out=xb, in_=_bcast_row(x16_d[b:b + 1, :]))
            uth = [ut]
        unit(b, uth, xb)

    # --- S0 via PE: S0[o, b] = sum_i w[o,i] x[b,i] (feeds only the combine,
    # so it is emitted last and fills engine idle time near the tail) ---
    wt = singles.tile([128, IN // 128, O_SH], F16)
    nc.sync.dma_start(out=wt, in_=wt16_d.rearrange("(ki p) o -> p ki o", p=128))
    xt = singles.tile([128, IN // 128, B], F16)
    nc.sync.dma_start(out=xt, in_=xt16_d.rearrange("(ki p) b -> p ki b", p=128))
    with tc.psum_pool(name="ps", bufs=2) as psp:
        for h in range(H):
            osl = slice(h * 128, (h + 1) * 128)
            pm = psp.tile([128, B], F32)
            for ki in range(IN // 128):
                nc.tensor.matmul(pm, wt[:, ki, osl], xt[:, ki, :],
                                 start=(ki == 0), stop=(ki == IN // 128 - 1))
            nc.vector.tensor_copy(s0[:, h * B:(h + 1) * B], pm)

    # --- final combine: out = accM (+ accX for b=0) + (0.5*S0 + bias) ---
    # biasS = 0.5*S0 + bias and the b=0 accX fold depend only on early work,
    # so they run long before the end; the tail is one tiny add + DMA
    biasS = singles.tile([128, H * B], F32)
    for h in range(H):
        nc.vector.tensor_scalar(
            biasS[:, h * B:(h + 1) * B], s0[:, h * B:(h + 1) * B],
            0.5, bias_col[:, h:h + 1], op.mult, op.add,
        )
        nc.vector.tensor_add(biasS[:, h * B:h * B + 1],
                             biasS[:, h * B:h * B + 1], accX[:, h:h + 1])
    comb = singles.tile([128, H * B], F32)
    nc.vector.tensor_add(comb, accM, biasS)
    out_v = out_d.rearrange("p h b -> p (h b)")
    nc.sync.dma_start(out=out_v, in_=comb)


def kernel(x, u, weight, log_alpha, bias):
    x = np.ascontiguousarray(x, dtype=np.float32)
    u = np.ascontiguousarray(u, dtype=np.float32)
    weight = np.ascontiguousarray(weight, dtype=np.float32)
    log_alpha = np.ascontiguousarray(log_alpha, dtype=np.float32)
    bias = np.ascontiguousarray(bias, dtype=np.float32)

    nc = _build_nc()

    x16 = x.astype(np.float16)
    in_maps = []
    for c in range(N_CORES):
        sl = slice(c * O_SH, (c + 1) * O_SH)
        wsl = weight[sl]
        in_maps.append(
            {
                "u": np.ascontiguousarray(u[:, sl, :]),
                "x16": x16,
                "w12": np.ascontiguousarray((1.2 * wsl).astype(np.float16)),
                "la16": np.ascontiguousarray(log_alpha[sl].astype(np.float16)),
                "wt16": np.ascontiguousarray(wsl.T.astype(np.float16)),
                "xt16": np.ascontiguousarray(x.T.astype(np.float16)),
                "bias": np.ascontiguousarray(bias[sl]),
            }
        )

    import os

    trace = bool(int(os.environ.get("KERNEL_TRACE", "0")))
    res = run_bass_kernel_spmd(
        nc, in_maps, core_ids=list(range(N_CORES)), trace=trace
    )
    kernel._last = res

    out = np.empty((B, OUT), dtype=np.float32)
    for c in range(N_CORES):
        oc = res.results[c]["out"]          # [128, H, B]
        out[:, c * O_SH:(c + 1) * O_SH] = oc.transpose(2, 1, 0).reshape(B, O_SH)
    return out


# revision 44
# speedup vs baseline: 1.0111x; 1.0051x over previous
"""Trainium2 Bass kernel for L0-regularized linear forward (hard-concrete gate).

Computes out[b,o] = sum_i x[b,i] * W[o,i] * z[b,o,i] + bias[o]
  where s = sigmoid((log(u) - log1p(-u) + log_alpha) / (2/3))
        z = clip(s * 1.2 - 0.1, 0, 1)

Shapes: x[32,2048] u[32,2048,2048] W[2048,2048] la[2048,2048] bias[2048]
Sharding: output-dim sharded, 2048/8 = 256 rows per core; no collectives.

Math used on device:
  y  = ln(u) - ln(1-u) + la;  yc = clamp(y, +-c), c = ln(11)/1.5
  z  = 1.2*sigmoid(1.5*yc) - 0.1              (exact: clip folds into clamp)
  sigmoid(1.5*y)-0.5 ~= p(y) = (c3*y^2 + c1)*y (deg-3, coefficients optimized
  for end-to-end variance against u~U(0,1); rel out err ~6e-3 vs 2e-2 gate)
  out = sum_i p(yc)*cb + 0.5*S0 + bias   with cb = 1.2*w*x, S0 = sum_i w*x

Engine schedule per (batch, half-of-256-rows) unit, all f16 after ACT:
  DMA : u half-tile (f32), broadcast row of x (f16)
  ACT : l2 = ln(1-u), l1 = ln(u)       (only the natural_log table -> 1 load)
  Pool: y1 = la - l2                   (scalar_tensor_tensor, eff 0.6)
  DVE : y = l1 + y1; custom op HC_GATE3_RED = clamp+poly3+mul-cb+accumulate
        in ONE instruction; cb = w12*xb (some offloaded to Pool)
  PE  : S0 row-sums of w*x (tiny f16 matmuls into PSUM)
"""

import sys
from contextlib import ExitStack

import numpy as np

if "/opt/trn_rl_repo" not in sys.path:
    sys.path.insert(0, "/opt/trn_rl_repo")

import concourse.bass as bass
import concourse.tile as tile
from concourse import bacc, mybir
from concourse.bass_utils import run_bass_kernel_spmd

F32 = mybir.dt.float32
F16 = mybir.dt.float16

B, OUT, IN = 32, 2048, 2048
N_CORES = 8
O_SH = OUT // N_CORES          # 256 output rows per core
H = O_SH // 128                # 2 partition-halves per core

C_CLAMP = float(np.log(11.0) / 1.5)
# variance-optimal deg-3: sigmoid(1.5 y) - 0.5 ~ (C3 y^2 + C1) y on [-c, c]
C1, C3 = 0.358500, -0.038292

N_CB_POOL = 3                  # of every 8 cb passes, this many go to Pool

_CACHE = {}


def _register_custom_op():
    """One fused DVE instruction:
        yc  = clamp(in0, -s0, s0)
        out = ((yc^2 * s1 + imm2) * yc) * in1
        accum_out = sum(out, free axis)
    8 ALU stages exactly (Zero-C0 is hoisted as stream-invariant)."""
    from concourse.dve_ops import (
        CUSTOM_DVE_SPECS,
        OPS,
        _CUSTOM_DVE_ROW_BASE,
        _SUB_OPCODE_FOR_NAME,
        DveOp,
    )
    from concourse.dve_spec import (
        AluOp, C0, Spec, Src0, Src1, Zero, lower, maxx, minn, sq,
    )
    from concourse.dve_spec import C1 as C1L, C2 as C2L
    from concourse.dve_table_gen import DveOpSpec

    name = "HC_GATE3_RED"
    if name in CUSTOM_DVE_SPECS:
        return next(o for o in OPS if o.name == name)

    yc = maxx(minn(Src0, C0), Zero - C0)
    body = ((sq(yc) * C1L + C2L) * yc) * Src1

    def _ref(in0, in1, s0, s1, imm2):
        x = np.clip(np.asarray(in0, np.float32), -s0, s0)
        w = np.asarray(in1, np.float32)
        out = ((x * x * s1 + imm2) * x) * w
        return out, out.sum(axis=-1, keepdims=True)

    spec = Spec(body=body, accum=AluOp.ADD, reference=_ref)
    row = _CUSTOM_DVE_ROW_BASE + len(OPS)
    _SUB_OPCODE_FOR_NAME[name] = row
    shas = {}
    for ver in ("v3", "v4"):
        dspec = DveOpSpec(name=name, opcode=row, uops=lower(spec, ver=ver),
                          rd1_en=True)
        shas[ver] = dspec.sha(ver)
    dve_op = DveOp(name, spec, subdim=False, uops_sha=shas)
    OPS.append(dve_op)
    CUSTOM_DVE_SPECS[name] = spec
    return dve_op


def _build_nc():
    if "nc" in _CACHE:
        return _CACHE["nc"]

    dve_op = _register_custom_op()

    nc = bacc.Bacc(
        "TRN2",
        target_bir_lowering=False,
        debug=False,
        num_devices=N_CORES,
    )
    u_d = nc.dram_tensor("u", [B, O_SH, IN], F32, kind="ExternalInput").ap()
    x16_d = nc.dram_tensor("x16", [B, IN], F16, kind="ExternalInput").ap()
    w12_d = nc.dram_tensor("w12", [O_SH, IN], F16, kind="ExternalInput").ap()
    la16_d = nc.dram_tensor("la16", [O_SH, IN], F16, kind="ExternalInput").ap()
    wt16_d = nc.dram_tensor("wt16", [IN, O_SH], F16, kind="ExternalInput").ap()
    xt16_d = nc.dram_tensor("xt16", [IN, B], F16, kind="ExternalInput").ap()
    bias_d = nc.dram_tensor("bias", [O_SH], F32, kind="ExternalInput").ap()
    out_d = nc.dram_tensor("out", [128, H, B], F32, kind="ExternalOutput").ap()

    with tile.TileContext(nc) as tc, ExitStack() as ctx:
        _kernel_body(ctx, tc, dve_op, u_d, x16_d, w12_d, la16_d, wt16_d,
                     xt16_d, bias_d, out_d)

    nc.compile()
    _CACHE["nc"] = nc
    return nc


def _bcast_row(ap_row):
    """[1, n] AP -> [128, n] AP with 0 partition stride."""
    return bass.AP(
        tensor=ap_row.tensor,
        offset=ap_row.offset,
        ap=[[0, 128], list(ap_row.ap[-1])],
    )


def _kernel_body(ctx, tc, dve_op, u_d, x16_d, w12_d, la16_d, wt16_d, xt16_d,
                 bias_d, out_d):
    nc = tc.nc
    Ln = mybir.ActivationFunctionType.Ln
    op = mybir.AluOpType

    singles = ctx.enter_context(tc.tile_pool(name="singles", bufs=1))

    # --- main loop pools (declared before constants so the first u/xb DMAs
    # are issued ahead of the setup DMAs on the in-order DMA queue) ---
    upool = ctx.enter_context(tc.tile_pool(name="u", bufs=3))
    l2pool = ctx.enter_context(tc.tile_pool(name="l2", bufs=2))
    ycpool = ctx.enter_context(tc.tile_pool(name="yc", bufs=2))
    xbpool = ctx.enter_context(tc.tile_pool(name="xb", bufs=2))
    cbpool = ctx.enter_context(tc.tile_pool(name="cb", bufs=4))
    junkpool = ctx.enter_context(tc.tile_pool(name="junk", bufs=2))

    # first working-set DMAs go out first; batch 0 is loaded per half so the
    # first ACT/DVE work starts after ~3us instead of ~6us of DMA
    ut0h = [upool.tile([128, IN], F32, name="uth") for h in range(H)]
    la16 = singles.tile([128, H, IN], F16)
    w12 = singles.tile([128, H, IN], F16)
    xb0 = xbpool.tile([128, IN], F16)
    la_v = la16_d.rearrange("(h p) i -> p h i", p=128)
    w12_v = w12_d.rearrange("(h p) i -> p h i", p=128)
    # dependency-ordered first transfers at quarter granularity: everything
    # the first (b=0, h=0, q=0) work needs, then successive quarters
    Q0 = IN // 2
    nc.sync.dma_start(out=ut0h[0][:, :Q0], in_=u_d[0, 0:128, :Q0])
    nc.sync.dma_start(out=la16[:, 0, :Q0], in_=la_v[:, 0, :Q0])
    nc.sync.dma_start(out=xb0[:, :Q0], in_=_bcast_row(x16_d[0:1, :Q0]))
    nc.sync.dma_start(out=w12[:, 0, :Q0], in_=w12_v[:, 0, :Q0])
    nc.sync.dma_start(out=ut0h[0][:, Q0:], in_=u_d[0, 0:128, Q0:])
    nc.sync.dma_start(out=la16[:, 0, Q0:], in_=la_v[:, 0, Q0:])
    nc.sync.dma_start(out=xb0[:, Q0:], in_=_bcast_row(x16_d[0:1, Q0:]))
    nc.sync.dma_start(out=w12[:, 0, Q0:], in_=w12_v[:, 0, Q0:])
    nc.sync.dma_start(out=ut0h[1], in_=u_d[0, 128:256, :])
    nc.sync.dma_start(out=la16[:, 1, :], in_=la_v[:, 1, :])
    nc.sync.dma_start(out=w12[:, 1, :], in_=w12_v[:, 1, :])
    bias_col = singles.tile([128, H], F32)
    nc.sync.dma_start(out=bias_col, in_=bias_d.rearrange("(h p) -> p h", p=128))

    # accumulator strip, column index = h*B + b
    accM = singles.tile([128, H * B], F32)
    accX = singles.tile([128, H], F32)      # b=0 second-column-half partials
    s0 = singles.tile([128, H * B], F32)

    def unit(b, uth, xb):
        # half-grain Ln for b==1 (its second u half is still in flight);
        # full-tile ops otherwise; y1/y-add always per-batch at 2x
        l2 = l2pool.tile([128, H, IN], F16)
        yc = ycpool.tile([128, H, IN], F16)
        if b in (1, 2):
            for h in range(H):
                nc.scalar.activation(l2[:, h, :], uth[h], Ln,
                                     bias=1.0, scale=-1.0)
                nc.vector.tensor_sub(l2[:, h, :], la16[:, h, :], l2[:, h, :])
                nc.scalar.activation(yc[:, h, :], uth[h], Ln)
                nc.vector.tensor_add(yc[:, h, :], yc[:, h, :], l2[:, h, :])
        else:
            ut = uth[0]
            nc.scalar.activation(l2, ut, Ln, bias=1.0, scale=-1.0)  # ln(1-u)
            nc.vector.tensor_sub(l2, la16, l2)
            nc.scalar.activation(yc, ut, Ln)                        # ln(u)
            nc.vector.tensor_add(yc, yc, l2)
        for h in range(H):
            cb = cbpool.tile([128, IN], F16)
            nc.gpsimd.tensor_mul(cb, w12[:, h, :], xb)           # Pool
            junk = junkpool.tile([128, IN], F16)
            col = h * B + b
            nc.vector._custom_dve(
                dve_op,
                out=junk,
                in0=yc[:, h, :],
                in1=cb,
                s0=C_CLAMP, s1=C3, imm2=C1,
                accum_out=accM[:, col:col + 1],
            )

    def unit0(uth, xb):
        # batch 0 runs at quarter granularity so every engine starts ~5us
        # earlier during pipeline fill; the second column-half accumulates
        # into accX and is folded in at the combine
        Q = IN // 2
        l2 = l2pool.tile([128, H, IN], F16)
        yc = ycpool.tile([128, H, IN], F16)
        for h in range(H):
            cb = cbpool.tile([128, IN], F16)
            junk = junkpool.tile([128, IN], F16)
            for q in range(2):
                cs = slice(q * Q, (q + 1) * Q)
                nc.scalar.activation(l2[:, h, cs], uth[h][:, cs], Ln,
                                     bias=1.0, scale=-1.0)
                nc.vector.tensor_sub(l2[:, h, cs], la16[:, h, cs], l2[:, h, cs])
                nc.scalar.activation(yc[:, h, cs], uth[h][:, cs], Ln)
                nc.vector.tensor_add(yc[:, h, cs], yc[:, h, cs], l2[:, h, cs])
                nc.gpsimd.tensor_mul(cb[:, cs], w12[:, h, cs], xb[:, cs])
                acc = accM[:, h * B:h * B + 1] if q == 0 else accX[:, h:h + 1]
                nc.vector._custom_dve(
                    dve_op,
                    out=junk[:, cs],
                    in0=yc[:, h, cs],
                    in1=cb[:, cs],
                    s0=C_CLAMP, s1=C3, imm2=C1,
                    accum_out=acc,
                )

    for b in range(B):
        if b == 0:
            unit0(ut0h, xb0)
            continue
        xb = xbpool.tile([128, IN], F16)
        if b in (1, 2):
            uth = [upool.tile([128, IN], F32, name="uth") for h in range(H)]
            nc.sync.dma_start(out=uth[0], in_=u_d[b, 0:128, :])
            nc.sync.dma_start(out=xb, in_=_bcast_row(x16_d[b:b + 1, :]))
            nc.sync.dma_start(out=uth[1], in_=u_d[b, 128:256, :])
        else:
            ut = upool.tile([128, H, IN], F32, name="utb", bufs=3)
            nc.sync.dma_start(out=ut, in_=u_d[b].rearrange("(h p) i -> p h i", p=128))
            nc.sync.dma_start(out=xb, in_=_bcast_row(x16_d[b:b + 1, :]))
            uth = [ut]
        unit(b, uth, xb)

    # --- S0 via PE: S0[o, b] = sum_i w[o,i] x[b,i] (feeds only the combine,
    # so it is emitted last and fills engine idle time near the tail) ---
    wt = singles.tile([128, IN // 128, O_SH], F16)
    nc.sync.dma_start(out=wt, in_=wt16_d.rearrange("(ki p) o -> p ki o", p=128))
    xt = singles.tile([128, IN // 128, B], F16)
    nc.sync.dma_start(out=xt, in_=xt16_d.rearrange("(ki p) b -> p ki b", p=128))
    with tc.psum_pool(name="ps", bufs=2) as psp:
        for h in range(H):
            osl = slice(h * 128, (h + 1) * 128)
            pm = psp.tile([128, B], F32)
            for ki in range(IN // 128):
                nc.tensor.matmul(pm, wt[:, ki, osl], xt[:, ki, :],
                                 start=(ki == 0), stop=(ki == IN // 128 - 1))
            nc.vector.tensor_copy(s0[:, h * B:(h + 1) * B], pm)

    # --- final combine: out = accM (+ accX for b=0) + (0.5*S0 + bias) ---
    # biasS = 0.5*S0 + bias and the b=0 accX fold depend only on early work,
    # so they run long before the end; the tail is one tiny add + DMA
    biasS = singles.tile([128, H * B], F32)
    for h in range(H):
        nc.vector.tensor_scalar(
            biasS[:, h * B:(h + 1) * B], s0[:, h * B:(h + 1) * B],
            0.5, bias_col[:, h:h + 1], op.mult, op.add,
        )
        nc.vector.tensor_add(biasS[:, h * B:h * B + 1],
                             biasS[:, h * B:h * B + 1], accX[:, h:h + 1])
    comb = singles.tile([128, H * B], F32)
    nc.vector.tensor_add(comb, accM, biasS)
    out_v = out_d.rearrange("p h b -> p (h b)")
    nc.sync.dma_start(out=out_v, in_=comb)


def kernel(x, u, weight, log_alpha, bias):
    x = np.ascontiguousarray(x, dtype=np.float32)
    u = np.ascontiguousarray(u, dtype=np.float32)
    weight = np.ascontiguousarray(weight, dtype=np.float32)
    log_alpha = np.ascontiguousarray(log_alpha, dtype=np.float32)
    bias = np.ascontiguousarray(bias, dtype=np.float32)

    nc = _build_nc()

    x16 = x.astype(np.float16)
    in_maps = []
    for c in range(N_CORES):
        sl = slice(c * O_SH, (c + 1) * O_SH)
        wsl = weight[sl]
        in_maps.append(
            {
                "u": np.ascontiguousarray(u[:, sl, :]),
                "x16": x16,
                "w12": np.ascontiguousarray((1.2 * wsl).astype(np.float16)),
                "la16": np.ascontiguousarray(log_alpha[sl].astype(np.float16)),
                "wt16": np.ascontiguousarray(wsl.T.astype(np.float16)),
                "xt16": np.ascontiguousarray(x.T.astype(np.float16)),
                "bias": np.ascontiguousarray(bias[sl]),
            }
        )

    import os

    trace = bool(int(os.environ.get("KERNEL_TRACE", "0")))
    res = run_bass_kernel_spmd(
        nc, in_maps, core_ids=list(range(N_CORES)), trace=trace
    )
    kernel._last = res

    out = np.empty((B, OUT), dtype=np.float32)
    for c in range(N_CORES):
        oc = res.results[c]["out"]          # [128, H, B]
        out[:, c * O_SH:(c + 1) * O_SH] = oc.transpose(2, 1, 0).reshape(B, O_SH)
    return out
